# revision 23
# baseline (speedup 1.0000x reference)
"""Two-layer GCN (PyG GCNConv defaults) on 8 Trainium2 NeuronCores.

Strategy (graph/data parallel, per sharding hint):
  - Destination nodes sharded 8 x 12800 (N=100000 padded to 102400), with a
    host-side balanced assignment of nodes to (core, block) bins so that
    every (dst-block, src-bucket) segment holds <= cap (target 512) edges.
    Buckets are within-shard quarters (25 blocks each); nodes are kept inside
    their home bucket's block range so src bucket ids are known before the
    permutation exists.  The output is un-permuted on the host.
  - Per layer: g = deg^{-1/2} * (h @ W) computed on each core's own shard
    (layer 1 folds deg^{-1/2} into x on the host), then AllGather'd per
    bucket (4 chunked collectives, bf16) so the layer-2 collectives overlap
    the layer-1 aggregation tail.
  - Edge aggregation per core: edges with dst in the core's shard, sorted by
    (piece, src-bucket, dst-block).  Source features fetched with dma_gather
    (int16 indices into the 25600-row bucket tables); the 4 buckets map to
    the 4 SWDGE queues with 3 pieces in flight per queue so all four queues'
    descriptor pipelines stay saturated (the gather is descriptor-rate-bound
    at ~7.9ns/desc/queue).
  - segment_sum realized as selector matmuls: S.T[e, d] = (dstloc[e] == d)
    built on-chip via is_equal against an iota tile, one batched is_equal per
    (piece, bucket); each dst block's four bucket-segments accumulate in one
    PSUM slice, finalized in place:
    out = deg^{-1/2} * (segsum + g_own) + b, LeakyReLU(0.01).  Layer 2 reuses
    the same edge structure against the layer-2 g tables.
  - All DRAM bounce buffers are partition-major so every per-piece DMA is a
    contiguous >=1.25KB run per partition (no descriptor storms競 with the
    SWDGE gather descriptors).
"""
import sys

sys.path.insert(0, "/opt/trn_rl_repo")

import numpy as np
import ml_dtypes

import concourse.bacc as bacc
import concourse.mybir as mybir
import concourse.tile as tile
from concourse.bass_utils import run_bass_kernel_spmd

NCORES = 8
N = 100000
E = 1600000
D = 128
BLK = 100               # dst blocks per core
SH = BLK * 128          # dst shard per core (12800)
NP_ = SH * NCORES       # padded node count 102400
NB = 4                  # src buckets = within-shard quarters (25 blocks)
QB = BLK // NB          # blocks per bucket-quarter (25)
BKT = NP_ // NB         # 25600 rows per bucket table
PIECE = 5               # dst blocks per dma_gather piece (100 = 20 * 5)
NPIECE = BLK // PIECE   # 20 pieces
NEG = 0.01

fp32 = mybir.dt.float32
bf16 = mybir.dt.bfloat16
i16 = mybir.dt.int16
i32 = mybir.dt.int32

_CACHE = {}


def _build(cap):
    ck = cap // 128                 # chunks per segment
    ts = NB * BLK * cap             # idx stream length per core / layer
    nch = ts // 128                 # total chunks in stream
    pidx = PIECE * cap              # idxs per gather piece per bucket
    pck = PIECE * ck                # chunks per (piece, bucket)

    nc = bacc.Bacc("TRN2", num_devices=NCORES, num_swdge_queues=4,
                   dynamic_dma_scratch_size=32768)
    xT_in = nc.dram_tensor("xT", [128, SH], bf16, kind="ExternalInput")
    w1_in = nc.dram_tensor("w1", [128, 128], fp32, kind="ExternalInput")
    w2_in = nc.dram_tensor("w2", [128, 128], fp32, kind="ExternalInput")
    b1_in = nc.dram_tensor("b1t", [128, 128], fp32, kind="ExternalInput")
    b2_in = nc.dram_tensor("b2t", [128, 128], fp32, kind="ExternalInput")
    cnt_in = nc.dram_tensor("cnt", [128, BLK], i32, kind="ExternalInput")
    sqd_in = nc.dram_tensor("sqd", [1, SH], bf16, kind="ExternalInput")
    idx_in = nc.dram_tensor("idx", [128, ts // 16], i16, kind="ExternalInput")
    dl_in = nc.dram_tensor("dstloc", [128, nch], bf16, kind="ExternalInput")
    iota_in = nc.dram_tensor("iota", [128, PIECE * cap], bf16,
                             kind="ExternalInput")
    ident_in = nc.dram_tensor("ident", [128, 128], bf16, kind="ExternalInput")
    # partition-major output: out2[p, c*128 + f] = out[c*128 + p, f]
    out_t = nc.dram_tensor("out", [128, SH], fp32, kind="ExternalOutput")

    with tile.TileContext(nc) as tc:
        with (
            tc.tile_pool(name="const", bufs=1) as cpool,
            tc.tile_pool(name="xchunk", bufs=3) as xpool,
            tc.tile_pool(name="msg", bufs=4) as mpool,
            tc.tile_pool(name="st", bufs=2) as stpool,
            tc.tile_pool(name="fin", bufs=3) as fpool,
            tc.tile_pool(name="ps_a", bufs=2, space="PSUM") as ps_a,
            tc.tile_pool(name="ps_w", bufs=1, space="PSUM") as ps_w,
            tc.tile_pool(name="ps_seg", bufs=4, space="PSUM") as ps_seg,
            tc.tile_pool(name="ps_t", bufs=1, space="PSUM") as ps_t,
            tc.tile_pool(name="dram", bufs=1, space="DRAM") as dram,
        ):
            # ---- resident constants -------------------------------------
            w1f = cpool.tile([128, 128], fp32)
            nc.sync.dma_start(w1f[:], w1_in[:])
            w1b = cpool.tile([128, 128], bf16)
            nc.vector.tensor_copy(w1b[:], w1f[:])
            w2f = cpool.tile([128, 128], fp32)
            nc.sync.dma_start(w2f[:], w2_in[:])
            w2b = cpool.tile([128, 128], bf16)
            nc.vector.tensor_copy(w2b[:], w2f[:])
            b1t = cpool.tile([128, 128], fp32)
            nc.sync.dma_start(b1t[:], b1_in[:])
            b2t = cpool.tile([128, 128], fp32)
            nc.sync.dma_start(b2t[:], b2_in[:])
            iota = cpool.tile([128, PIECE * cap], bf16)
            nc.sync.dma_start(iota[:], iota_in[:])
            ident = cpool.tile([128, 128], bf16)
            nc.sync.dma_start(ident[:], ident_in[:])
            dstloc = cpool.tile([128, nch], bf16)
            nc.sync.dma_start(dstloc[:], dl_in[:])
            # full idx stream resident (shared by both layers)
            ixall = cpool.tile([128, ts // 16], i16)
            nc.sync.dma_start(ixall[:], idx_in[:])

            # deg^{-1/2} from int32 counts:  1 / sqrt(cnt + 1)
            cnts = cpool.tile([128, BLK], i32)
            nc.sync.dma_start(cnts[:], cnt_in[:])
            degf = cpool.tile([128, BLK], fp32)
            nc.vector.tensor_copy(degf[:], cnts[:])
            sq = cpool.tile([128, BLK], fp32)
            nc.scalar.activation(sq[:], degf[:],
                                 mybir.ActivationFunctionType.Sqrt, bias=1.0)
            dq = cpool.tile([128, BLK], fp32)
            nc.vector.reciprocal(dq[:], sq[:])

            b1r = cpool.tile([1, 128], bf16)
            nc.vector.tensor_copy(b1r[:], b1t[0:1, :])
            b2r = cpool.tile([1, 128], bf16)
            nc.vector.tensor_copy(b2r[:], b2t[0:1, :])

            # DRAM bounce (partition-major, one per bucket-quarter) + tables
            g1_bq = [dram.tile([128, QB * 128], bf16, name=f"g1_bq{r}")
                     for r in range(NB)]
            g2_bq = [dram.tile([128, QB * 128], bf16, name=f"g2_bq{r}")
                     for r in range(NB)]
            g1_t = [dram.tile([BKT, D], bf16, name=f"g1_t{r}",
                              addr_space="Shared") for r in range(NB)]
            g2_t = [dram.tile([BKT, D], bf16, name=f"g2_t{r}",
                              addr_space="Shared") for r in range(NB)]

            def cc(src_bq, dst_t):
                # AllGather output is the raw core-major concat of the 8
                # [128, QB*128] partition-major shard buffers; viewed as a
                # [BKT, 128] table, row (c*128+p)*QB + a holds the features
                # of (core c, quarter-block a, slot p).
                nc.gpsimd.collective_compute(
                    "AllGather", mybir.AluOpType.bypass,
                    replica_groups=[list(range(NCORES))],
                    ins=[src_bq[:].opt()],
                    outs=[dst_t[:].opt()])

            # ---- phase A: g1 = (dq*x) @ W1 on own shard (dq host-folded,
            # x pre-cast to bf16).  Casts on Scalar, stores on GpSimd so the
            # Vector queue stays free for the selector prefetch.
            with nc.named_scope("phaseA"):
                for r in range(NB):
                    for i in range(0, QB, 4):
                        w = min(4, QB - i)
                        c0 = (r * QB + i) * 128
                        xc = xpool.tile([128, 512], bf16, tag="xc")
                        nc.sync.dma_start(xc[:, :w * 128],
                                          xT_in[:, c0:c0 + w * 128])
                        psA = ps_a.tile([128, 512], fp32, space="PSUM",
                                        tag="psA")
                        for j in range(w):
                            nc.tensor.matmul(
                                out=psA[:, j * 128:(j + 1) * 128],
                                lhsT=xc[:, j * 128:(j + 1) * 128],
                                rhs=w1b[:], start=True, stop=True)
                        gtb = xpool.tile([128, 512], bf16, tag="gtb")
                        nc.scalar.activation(
                            gtb[:, :w * 128], psA[:, :w * 128],
                            mybir.ActivationFunctionType.Copy)
                        nc.scalar.dma_start(
                            g1_bq[r][:, i * 128:(i + w) * 128],
                            gtb[:, :w * 128])
                    with nc.named_scope(f"cc1_{r}"):
                        cc(g1_bq[r], g1_t[r])

            # ---- aggregation over edges (shared for both layers) --------
            def aggregate(g_t, g_bq, b_row, finalize, stage_dt, flush,
                          after_piece):
                for p in range(NPIECE):
                    msgs = []
                    for r in range(NB):
                        i0 = (p * NB + r) * pidx // 16
                        msg = mpool.tile([128, pck, 128], bf16, tag=f"msg{r}")
                        nc.gpsimd.dma_gather(
                            msg[:], g_t[r][:, :],
                            ixall[:, i0:i0 + pidx // 16],
                            pidx, pidx, 128, single_packet=False,
                            queue_num=r)
                        msgs.append(msg)
                    # own-shard rows for the self-loop term
                    gown = fpool.tile([128, PIECE * 128], bf16, tag="gown")
                    nc.sync.dma_start(
                        gown[:],
                        g_bq[p // PIECE][:, (p % PIECE) * PIECE * 128:
                                         (p % PIECE + 1) * PIECE * 128])
                    # per-piece sqrt(deg) row for the rank-1 bias fold
                    sqp = fpool.tile([1, PIECE * 128], bf16, tag="sqp")
                    nc.sync.dma_start(
                        sqp[:],
                        sqd_in[0:1, p * PIECE * 128:(p + 1) * PIECE * 128])
                    # batched selector build: one is_equal per (piece,bucket)
                    sts = []
                    for r in range(NB):
                        gc0 = (p * NB + r) * pck
                        st = stpool.tile([128, pck, 128], bf16, tag=f"st{r}")
                        nc.vector.tensor_tensor(
                            st[:],
                            iota[:].rearrange("q (c j) -> q c j", c=pck),
                            dstloc[:, gc0:gc0 + pck].to_broadcast(
                                [128, pck, 128]),
                            mybir.AluOpType.is_equal)
                        sts.append(st)
                    stage = fpool.tile([128, PIECE * 128], stage_dt,
                                       tag="stage")
                    for s in range(PIECE):
                        c = p * PIECE + s
                        pseg = ps_seg.tile([128, 128], fp32, space="PSUM",
                                           tag="pseg")
                        for r in range(NB):
                            for k in range(ck):
                                nc.tensor.matmul(
                                    out=pseg[:],
                                    lhsT=sts[r][:, s * ck + k, :],
                                    rhs=msgs[r][:, s * ck + k, :],
                                    start=(r == 0 and k == 0),
                                    stop=False)
                        # self-loop: pseg += I.T @ g_own ;  bias (pre-scale):
                        # pseg += sqrt(deg) (x) b  so that (pseg)*dq = out+b
                        nc.tensor.matmul(out=pseg[:], lhsT=ident[:],
                                         rhs=gown[:, s * 128:(s + 1) * 128],
                                         start=False, stop=False)
                        nc.tensor.matmul(out=pseg[:],
                                         lhsT=sqp[0:1, s * 128:(s + 1) * 128],
                                         rhs=b_row[:],
                                         start=False, stop=True)
                        finalize(c, pseg[:], stage[:, s * 128:(s + 1) * 128])
                    flush(p, stage)
                    after_piece(p, msgs)

            # ---- layer-1 finalize: h2 path, produces g2 -----------------
            def fin1(c, pslice, sl):
                r1 = fpool.tile([128, 128], bf16, tag="r1")
                nc.scalar.activation(r1[:], pslice,
                                     mybir.ActivationFunctionType.Lrelu,
                                     scale=dq[:, c:c + 1], alpha=NEG)
                ptr = ps_t.tile([128, 128], bf16, space="PSUM", tag="ptr")
                nc.tensor.transpose(ptr[:], r1[:], ident[:])
                r1T = fpool.tile([128, 128], bf16, tag="r1T")
                nc.vector.tensor_copy(r1T[:], ptr[:])
                ph2 = ps_w.tile([128, 128], fp32, space="PSUM", tag="ph")
                nc.tensor.matmul(out=ph2[:], lhsT=r1T[:], rhs=w2b[:],
                                 start=True, stop=True)
                nc.scalar.activation(sl, ph2[:],
                                     mybir.ActivationFunctionType.Copy,
                                     scale=dq[:, c:c + 1])

            def flush1(p, stage):
                nc.sync.dma_start(
                    g2_bq[p // PIECE][:, (p % PIECE) * PIECE * 128:
                                      (p % PIECE + 1) * PIECE * 128],
                    stage[:])

            # layer-2 collectives run serially after agg1 finishes gathering:
            # overlapping them with the gathers just splits the shared
            # DMA-engine time (both streams slow ~proportionally) while
            # fragmenting the gather descriptor pipeline.  The dummy GpSimd
            # reads of the final msg tiles gate the collective dispatches
            # behind the last gather transfers (GpSimd issues run far ahead
            # of the data otherwise).
            def after1(p, msgs):
                if p == NPIECE - 1:
                    for r in range(NB):
                        gate = fpool.tile([128, 16], bf16, tag=f"gate{r}")
                        nc.gpsimd.tensor_copy(gate[:], msgs[r][:, 0, 0:16])
                    for r in range(NB):
                        with nc.named_scope(f"cc2_{r}"):
                            cc(g2_bq[r], g2_t[r])

            def fin2(c, pslice, sl):
                nc.scalar.activation(sl, pslice,
                                     mybir.ActivationFunctionType.Lrelu,
                                     scale=dq[:, c:c + 1], alpha=NEG)

            def flush2(p, stage):
                nc.sync.dma_start(
                    out_t[:, p * PIECE * 128:(p + 1) * PIECE * 128], stage[:])

            def after2(p, msgs):
                pass

            with nc.named_scope("agg1"):
                aggregate(g1_t, g1_bq, b1r, fin1, bf16, flush1, after1)

            with nc.named_scope("agg2"):
                aggregate(g2_t, g2_bq, b2r, fin2, fp32, flush2, after2)

    nc.compile()
    return nc


def _balance(src, dst):
    """Choose the node -> global-slot permutation.

    1. Home buckets: partition nodes into NB groups (<= BKT nodes each)
       equalizing total OUT-edge counts.
    2. Per home bucket, greedily pack its nodes into its 200 bins (the
       bucket's QB blocks on each of the 8 cores, 128 slots each), balancing
       the bins' per-src-bucket IN-edge loads with a hard cap of 512.
    Returns (perm, loads) where perm[v] = global slot.
    """
    outdeg = np.bincount(src, minlength=N).astype(np.int64)
    order = np.argsort(-outdeg, kind="stable")
    bket = np.full(N, -1, np.int8)
    nodes_per_bucket = N // NB
    for r in range(NB):
        bket[order[r::NB][:nodes_per_bucket]] = r

    bucket_of_src = bket[src]
    degb = np.zeros((N, NB), np.int64)
    np.add.at(degb, (dst, bucket_of_src), 1)

    CAP = 512
    NPB = 128
    NBINS = NCORES * QB  # 200 bins per bucket group
    perm = np.full(N, -1, np.int64)
    all_loads = np.zeros((NCORES * BLK, NB), np.int64)
    for q in range(NB):
        nodes_q = np.where(bket == q)[0]
        nodes_q = nodes_q[np.argsort(-degb[nodes_q].sum(1), kind="stable")]
        loads = np.zeros((NBINS, NB), np.int64)
        slots = np.zeros(NBINS, np.int64)
        members = [[] for _ in range(NBINS)]
        for v in nodes_q:
            d = degb[v]
            cand = loads + d
            worst = cand.max(1)
            feas = (slots < NPB) & (cand <= CAP).all(1)
            if feas.any():
                score = np.where(feas, worst * 1000 + slots, 1 << 50)
            else:
                score = np.where(slots < NPB, worst * 1000 + slots, 1 << 50)
            b = int(np.argmin(score))
            members[b].append(v)
            loads[b] += d
            slots[b] += 1
        for b in range(NBINS):
            core, j = divmod(b, QB)
            gbin = core * BLK + q * QB + j
            base = gbin * 128
            for i, v in enumerate(members[b]):
                perm[v] = base + i
            all_loads[gbin] = loads[b]
    return perm, all_loads


def _preprocess(x, edge_index, cnt_only=False):
    src = np.asarray(edge_index[0], dtype=np.int64)
    dst = np.asarray(edge_index[1], dtype=np.int64)

    perm, loads = _balance(src, dst)
    cap = max(512, int(-(-loads.max() // 128) * 128))

    gsrc = perm[src]
    gdst = perm[dst]
    core = gdst // SH
    block = (gdst % SH) // 128
    dstloc = (gdst % 128).astype(np.float32)
    bucket = (gsrc % SH) // (QB * 128)
    # bucket table row: cores-major [8*128, QB, 128]-bf16 layout
    srcloc = ((gsrc // SH) * 128 + gsrc % 128) * QB \
        + ((gsrc % SH) // 128) % QB
    srcloc = srcloc.astype(np.int16)

    pc = block // PIECE
    sp = block % PIECE
    seg = ((core * NPIECE + pc) * NB + bucket) * PIECE + sp
    counts = np.bincount(seg, minlength=NCORES * NB * BLK)
    assert counts.max() <= cap, (counts.max(), cap)

    order = np.argsort(seg, kind="stable")
    seg_s = seg[order]
    starts = np.zeros(NCORES * NB * BLK + 1, np.int64)
    np.cumsum(counts, out=starts[1:])
    pos = np.arange(E, dtype=np.int64) - starts[seg_s]
    slot = seg_s * cap + pos

    total = NCORES * NB * BLK * cap
    idx_arr = np.zeros(total, np.int16)
    idx_arr[slot] = srcloc[order]
    dl_arr = np.full(total, 255.0, np.float32)
    dl_arr[slot] = dstloc[order]
    ts = NB * BLK * cap
    idx_arr = idx_arr.reshape(NCORES, ts)
    dl_arr = dl_arr.reshape(NCORES, ts)

    cnt = np.bincount(gdst, minlength=NP_).astype(np.int32)

    # host-folded deg^{-1/2} scaling of x for layer 1, pre-cast to bf16
    x = np.asarray(x, np.float32)
    dqv = 1.0 / np.sqrt(cnt[perm].astype(np.float64) + 1.0)
    xpad = np.zeros((NP_, D), ml_dtypes.bfloat16)
    xpad[perm] = (x * dqv[:, None].astype(np.float32)).astype(
        ml_dtypes.bfloat16)

    iota = np.tile(np.arange(128, dtype=np.float32),
                   (128, PIECE * cap // 128)).astype(ml_dtypes.bfloat16)
    ident = np.eye(128, dtype=ml_dtypes.bfloat16)

    return cap, perm, idx_arr, dl_arr, cnt, xpad, iota, ident


def kernel(x, W1, b1, W2, b2, edge_index, batch):
    x = np.asarray(x, np.float32)
    W1 = np.asarray(W1, np.float32)
    W2 = np.asarray(W2, np.float32)
    b1 = np.asarray(b1, np.float32)
    b2 = np.asarray(b2, np.float32)

    cap, perm, idx_arr, dl_arr, cnt, xpad, iota, ident = \
        _preprocess(x, edge_index)

    if cap not in _CACHE:
        _CACHE[cap] = _build(cap)
    nc = _CACHE[cap]

    b1t = np.tile(b1, (128, 1))
    b2t = np.tile(b2, (128, 1))
    in_maps = []
    for c in range(NCORES):
        sl = slice(c * SH, (c + 1) * SH)
        wrapped = np.tile(idx_arr[c].reshape(-1, 16).T, (8, 1))
        in_maps.append({
            "xT": np.ascontiguousarray(xpad[sl].T),
            "w1": W1, "w2": W2, "b1t": b1t, "b2t": b2t,
            "cnt": np.ascontiguousarray(cnt[sl].reshape(BLK, 128).T),
            "sqd": np.ascontiguousarray(
                np.sqrt(cnt[sl][None, :].astype(np.float64) + 1.0)
                .astype(ml_dtypes.bfloat16)),
            "idx": np.ascontiguousarray(wrapped),
            "dstloc": np.ascontiguousarray(
                dl_arr[c].reshape(-1, 128).T.astype(ml_dtypes.bfloat16)),
            "iota": iota, "ident": ident,
        })

    import os
    trace = bool(os.environ.get("KERNEL_TRACE"))
    rr = run_bass_kernel_spmd(nc, in_maps, list(range(NCORES)), trace=trace)
    if trace:
        kernel.last_results = rr
    # out[p, c*128+f] per core -> rows c*128+p
    outs = []
    for c in range(NCORES):
        o = rr.results[c]["out"].reshape(128, BLK, 128)
        outs.append(np.transpose(o, (1, 0, 2)).reshape(SH, 128))
    out = np.concatenate(outs, axis=0)
    return np.ascontiguousarray(out[perm])


# revision 25
# speedup vs baseline: 1.0141x; 1.0141x over previous
"""Two-layer GCN (PyG GCNConv defaults) on 8 Trainium2 NeuronCores.

Strategy (graph/data parallel, per sharding hint):
  - Destination nodes sharded 8 x 12800 (N=100000 padded to 102400), with a
    host-side balanced assignment of nodes to (core, block) bins so that
    every (dst-block, src-bucket) segment holds <= cap (target 512) edges.
    Buckets are within-shard quarters (25 blocks each); nodes are kept inside
    their home bucket's block range so src bucket ids are known before the
    permutation exists.  The output is un-permuted on the host.
  - Per layer: g = deg^{-1/2} * (h @ W) computed on each core's own shard
    (layer 1 folds deg^{-1/2} into x on the host), then AllGather'd per
    bucket (4 chunked collectives, bf16) so the layer-2 collectives overlap
    the layer-1 aggregation tail.
  - Edge aggregation per core: edges with dst in the core's shard, sorted by
    (piece, src-bucket, dst-block).  Source features fetched with dma_gather
    (int16 indices into the 25600-row bucket tables); the 4 buckets map to
    the 4 SWDGE queues with 3 pieces in flight per queue so all four queues'
    descriptor pipelines stay saturated (the gather is descriptor-rate-bound
    at ~7.9ns/desc/queue).
  - segment_sum realized as selector matmuls: S.T[e, d] = (dstloc[e] == d)
    built on-chip via is_equal against an iota tile, one batched is_equal per
    (piece, bucket); each dst block's four bucket-segments accumulate in one
    PSUM slice, finalized in place:
    out = deg^{-1/2} * (segsum + g_own) + b, LeakyReLU(0.01).  Layer 2 reuses
    the same edge structure against the layer-2 g tables.
  - All DRAM bounce buffers are partition-major so every per-piece DMA is a
    contiguous >=1.25KB run per partition (no descriptor storms競 with the
    SWDGE gather descriptors).
"""
import sys

sys.path.insert(0, "/opt/trn_rl_repo")

import numpy as np
import ml_dtypes

import concourse.bacc as bacc
import concourse.mybir as mybir
import concourse.tile as tile
from concourse.bass_utils import run_bass_kernel_spmd

NCORES = 8
N = 100000
E = 1600000
D = 128
BLK = 100               # dst blocks per core
SH = BLK * 128          # dst shard per core (12800)
NP_ = SH * NCORES       # padded node count 102400
NB = 4                  # src buckets = within-shard quarters (25 blocks)
QB = BLK // NB          # blocks per bucket-quarter (25)
BKT = NP_ // NB         # 25600 rows per bucket table
PIECE = 5               # dst blocks per dma_gather piece (100 = 20 * 5)
NPIECE = BLK // PIECE   # 20 pieces
NEG = 0.01

fp32 = mybir.dt.float32
bf16 = mybir.dt.bfloat16
i16 = mybir.dt.int16
i32 = mybir.dt.int32

_CACHE = {}


def _build(cap):
    ck = cap // 128                 # chunks per segment
    ts = NB * BLK * cap             # idx stream length per core / layer
    nch = ts // 128                 # total chunks in stream
    pidx = PIECE * cap              # idxs per gather piece per bucket
    pck = PIECE * ck                # chunks per (piece, bucket)

    nc = bacc.Bacc("TRN2", num_devices=NCORES, num_swdge_queues=4,
                   dynamic_dma_scratch_size=32768)
    xT_in = nc.dram_tensor("xT", [128, SH], bf16, kind="ExternalInput")
    w1_in = nc.dram_tensor("w1", [128, 128], fp32, kind="ExternalInput")
    w2_in = nc.dram_tensor("w2", [128, 128], fp32, kind="ExternalInput")
    b1_in = nc.dram_tensor("b1t", [128, 128], fp32, kind="ExternalInput")
    b2_in = nc.dram_tensor("b2t", [128, 128], fp32, kind="ExternalInput")
    cnt_in = nc.dram_tensor("cnt", [128, BLK], i32, kind="ExternalInput")
    sqd_in = nc.dram_tensor("sqd", [1, SH], bf16, kind="ExternalInput")
    idx_in = nc.dram_tensor("idx", [128, ts // 16], i16, kind="ExternalInput")
    dl_in = nc.dram_tensor("dstloc", [128, nch], bf16, kind="ExternalInput")
    iota_in = nc.dram_tensor("iota", [128, PIECE * cap], bf16,
                             kind="ExternalInput")
    ident_in = nc.dram_tensor("ident", [128, 128], bf16, kind="ExternalInput")
    # partition-major output: out2[p, c*128 + f] = out[c*128 + p, f]
    out_t = nc.dram_tensor("out", [128, SH], fp32, kind="ExternalOutput")

    with tile.TileContext(nc) as tc:
        with (
            tc.tile_pool(name="const", bufs=1) as cpool,
            tc.tile_pool(name="xchunk", bufs=3) as xpool,
            tc.tile_pool(name="msg", bufs=4) as mpool,
            tc.tile_pool(name="st", bufs=2) as stpool,
            tc.tile_pool(name="fin", bufs=3) as fpool,
            tc.tile_pool(name="ps_a", bufs=2, space="PSUM") as ps_a,
            tc.tile_pool(name="ps_w", bufs=1, space="PSUM") as ps_w,
            tc.tile_pool(name="ps_seg", bufs=4, space="PSUM") as ps_seg,
            tc.tile_pool(name="ps_t", bufs=1, space="PSUM") as ps_t,
            tc.tile_pool(name="dram", bufs=1, space="DRAM") as dram,
        ):
            # ---- resident constants -------------------------------------
            w1f = cpool.tile([128, 128], fp32)
            nc.sync.dma_start(w1f[:], w1_in[:])
            w1b = cpool.tile([128, 128], bf16)
            nc.vector.tensor_copy(w1b[:], w1f[:])
            w2f = cpool.tile([128, 128], fp32)
            nc.sync.dma_start(w2f[:], w2_in[:])
            w2b = cpool.tile([128, 128], bf16)
            nc.vector.tensor_copy(w2b[:], w2f[:])
            b1t = cpool.tile([128, 128], fp32)
            nc.sync.dma_start(b1t[:], b1_in[:])
            b2t = cpool.tile([128, 128], fp32)
            nc.sync.dma_start(b2t[:], b2_in[:])
            iota = cpool.tile([128, PIECE * cap], bf16)
            nc.sync.dma_start(iota[:], iota_in[:])
            ident = cpool.tile([128, 128], bf16)
            nc.sync.dma_start(ident[:], ident_in[:])
            dstloc = cpool.tile([128, nch], bf16)
            nc.sync.dma_start(dstloc[:], dl_in[:])
            # full idx stream resident (shared by both layers)
            ixall = cpool.tile([128, ts // 16], i16)
            nc.sync.dma_start(ixall[:], idx_in[:])

            # deg^{-1/2} from int32 counts:  1 / sqrt(cnt + 1)
            cnts = cpool.tile([128, BLK], i32)
            nc.sync.dma_start(cnts[:], cnt_in[:])
            degf = cpool.tile([128, BLK], fp32)
            nc.vector.tensor_copy(degf[:], cnts[:])
            sq = cpool.tile([128, BLK], fp32)
            nc.scalar.activation(sq[:], degf[:],
                                 mybir.ActivationFunctionType.Sqrt, bias=1.0)
            dq = cpool.tile([128, BLK], fp32)
            nc.vector.reciprocal(dq[:], sq[:])

            b1r = cpool.tile([1, 128], bf16)
            nc.vector.tensor_copy(b1r[:], b1t[0:1, :])
            b2r = cpool.tile([1, 128], bf16)
            nc.vector.tensor_copy(b2r[:], b2t[0:1, :])

            # DRAM bounce + gathered tables (row-major: table row = gslot,
            # so each bucket table is a contiguous slice -- one efficient
            # monolithic AllGather per layer)
            g1_b = dram.tile([SH, D], bf16, name="g1_b")
            g1_full = dram.tile([NP_, D], bf16, name="g1_full",
                                addr_space="Shared")
            g2_b = dram.tile([SH, D], bf16, name="g2_b")
            g2_full = dram.tile([NP_, D], bf16, name="g2_full",
                                addr_space="Shared")

            def cc(src_b, dst_t):
                nc.gpsimd.collective_compute(
                    "AllGather", mybir.AluOpType.bypass,
                    replica_groups=[list(range(NCORES))],
                    ins=[src_b[:].opt()],
                    outs=[dst_t[:].opt()])

            # ---- phase A: g1 = (dq*x) @ W1 on own shard (dq host-folded,
            # x pre-cast to bf16).  Casts on Scalar, stores on GpSimd so the
            # Vector queue stays free for the selector prefetch.
            with nc.named_scope("phaseA"):
                for i in range(0, BLK, 4):
                    w = min(4, BLK - i)
                    xc = xpool.tile([128, 512], bf16, tag="xc")
                    nc.sync.dma_start(xc[:, :w * 128],
                                      xT_in[:, i * 128:(i + w) * 128])
                    psA = ps_a.tile([128, 512], fp32, space="PSUM",
                                    tag="psA")
                    for j in range(w):
                        nc.tensor.matmul(
                            out=psA[:, j * 128:(j + 1) * 128],
                            lhsT=xc[:, j * 128:(j + 1) * 128],
                            rhs=w1b[:], start=True, stop=True)
                    gtb = xpool.tile([128, 4, 128], bf16, tag="gtb")
                    nc.scalar.activation(
                        gtb[:, :w, :], psA[:, :w * 128].rearrange(
                            "q (a d) -> q a d", a=w),
                        mybir.ActivationFunctionType.Copy)
                    nc.sync.dma_start(
                        g1_b[:].rearrange("(a p) d -> p a d", p=128)
                        [:, i:i + w, :], gtb[:, :w, :])
                with nc.named_scope("cc1"):
                    cc(g1_b, g1_full)

            # ---- aggregation over edges (shared for both layers) --------
            def aggregate(g_full, g_b, b_row, finalize, stage_dt, flush,
                          after_piece):
                for p in range(NPIECE):
                    msgs = []
                    for r in range(NB):
                        i0 = (p * NB + r) * pidx // 16
                        msg = mpool.tile([128, pck, 128], bf16, tag=f"msg{r}")
                        nc.gpsimd.dma_gather(
                            msg[:], g_full[r * BKT:(r + 1) * BKT, :],
                            ixall[:, i0:i0 + pidx // 16],
                            pidx, pidx, 128, single_packet=False,
                            queue_num=r)
                        msgs.append(msg)
                    # own-shard rows for the self-loop term
                    gown = fpool.tile([128, PIECE * 128], bf16, tag="gown")
                    nc.sync.dma_start(
                        gown[:].rearrange("q (a d) -> q a d", a=PIECE),
                        g_b[:].rearrange("(a p) d -> p a d", p=128)
                        [:, p * PIECE:(p + 1) * PIECE, :])
                    # per-piece sqrt(deg) row for the rank-1 bias fold
                    sqp = fpool.tile([1, PIECE * 128], bf16, tag="sqp")
                    nc.sync.dma_start(
                        sqp[:],
                        sqd_in[0:1, p * PIECE * 128:(p + 1) * PIECE * 128])
                    # batched selector build: one is_equal per (piece,bucket)
                    sts = []
                    for r in range(NB):
                        gc0 = (p * NB + r) * pck
                        st = stpool.tile([128, pck, 128], bf16, tag=f"st{r}")
                        nc.vector.tensor_tensor(
                            st[:],
                            iota[:].rearrange("q (c j) -> q c j", c=pck),
                            dstloc[:, gc0:gc0 + pck].to_broadcast(
                                [128, pck, 128]),
                            mybir.AluOpType.is_equal)
                        sts.append(st)
                    stage = fpool.tile([128, PIECE * 128], stage_dt,
                                       tag="stage")
                    for s in range(PIECE):
                        c = p * PIECE + s
                        pseg = ps_seg.tile([128, 128], fp32, space="PSUM",
                                           tag="pseg")
                        for r in range(NB):
                            for k in range(ck):
                                nc.tensor.matmul(
                                    out=pseg[:],
                                    lhsT=sts[r][:, s * ck + k, :],
                                    rhs=msgs[r][:, s * ck + k, :],
                                    start=(r == 0 and k == 0),
                                    stop=False)
                        # self-loop: pseg += I.T @ g_own ;  bias (pre-scale):
                        # pseg += sqrt(deg) (x) b  so that (pseg)*dq = out+b
                        nc.tensor.matmul(out=pseg[:], lhsT=ident[:],
                                         rhs=gown[:, s * 128:(s + 1) * 128],
                                         start=False, stop=False)
                        nc.tensor.matmul(out=pseg[:],
                                         lhsT=sqp[0:1, s * 128:(s + 1) * 128],
                                         rhs=b_row[:],
                                         start=False, stop=True)
                        finalize(c, pseg[:], stage[:, s * 128:(s + 1) * 128])
                    flush(p, stage)
                    after_piece(p, msgs)

            # ---- layer-1 finalize: h2 path, produces g2 -----------------
            def fin1(c, pslice, sl):
                r1 = fpool.tile([128, 128], bf16, tag="r1")
                nc.scalar.activation(r1[:], pslice,
                                     mybir.ActivationFunctionType.Lrelu,
                                     scale=dq[:, c:c + 1], alpha=NEG)
                ptr = ps_t.tile([128, 128], bf16, space="PSUM", tag="ptr")
                nc.tensor.transpose(ptr[:], r1[:], ident[:])
                r1T = fpool.tile([128, 128], bf16, tag="r1T")
                nc.vector.tensor_copy(r1T[:], ptr[:])
                ph2 = ps_w.tile([128, 128], fp32, space="PSUM", tag="ph")
                nc.tensor.matmul(out=ph2[:], lhsT=r1T[:], rhs=w2b[:],
                                 start=True, stop=True)
                nc.scalar.activation(sl, ph2[:],
                                     mybir.ActivationFunctionType.Copy,
                                     scale=dq[:, c:c + 1])

            def flush1(p, stage):
                nc.sync.dma_start(
                    g2_b[:].rearrange("(a p) d -> p a d", p=128)
                    [:, p * PIECE:(p + 1) * PIECE, :],
                    stage[:].rearrange("q (a d) -> q a d", a=PIECE))

            # layer-2 collectives run serially after agg1 finishes gathering:
            # overlapping them with the gathers just splits the shared
            # DMA-engine time (both streams slow ~proportionally) while
            # fragmenting the gather descriptor pipeline.  The dummy GpSimd
            # reads of the final msg tiles gate the collective dispatches
            # behind the last gather transfers (GpSimd issues run far ahead
            # of the data otherwise).
            def after1(p, msgs):
                if p == NPIECE - 1:
                    with nc.named_scope("cc2"):
                        cc(g2_b, g2_full)

            def fin2(c, pslice, sl):
                nc.scalar.activation(sl, pslice,
                                     mybir.ActivationFunctionType.Lrelu,
                                     scale=dq[:, c:c + 1], alpha=NEG)

            def flush2(p, stage):
                nc.sync.dma_start(
                    out_t[:, p * PIECE * 128:(p + 1) * PIECE * 128], stage[:])

            def after2(p, msgs):
                pass

            with nc.named_scope("agg1"):
                aggregate(g1_full, g1_b, b1r, fin1, bf16, flush1, after1)

            with nc.named_scope("agg2"):
                aggregate(g2_full, g2_b, b2r, fin2, fp32, flush2, after2)

    nc.compile()
    return nc


def _balance(src, dst):
    """Choose the node -> global-slot permutation.

    1. Home buckets: partition nodes into NB groups (<= BKT nodes each)
       equalizing total OUT-edge counts.
    2. Per home bucket, greedily pack its nodes into its 200 bins (the
       bucket's QB blocks on each of the 8 cores, 128 slots each), balancing
       the bins' per-src-bucket IN-edge loads with a hard cap of 512.
    Returns (perm, loads) where perm[v] = global slot.
    """
    outdeg = np.bincount(src, minlength=N).astype(np.int64)
    order = np.argsort(-outdeg, kind="stable")
    bket = np.full(N, -1, np.int8)
    nodes_per_bucket = N // NB
    for r in range(NB):
        bket[order[r::NB][:nodes_per_bucket]] = r

    bucket_of_src = bket[src]
    degb = np.zeros((N, NB), np.int64)
    np.add.at(degb, (dst, bucket_of_src), 1)

    CAP = 512
    NPB = 128
    NBINS = NCORES * QB  # 200 bins per bucket group
    perm = np.full(N, -1, np.int64)
    all_loads = np.zeros((NCORES * BLK, NB), np.int64)
    for q in range(NB):
        nodes_q = np.where(bket == q)[0]
        nodes_q = nodes_q[np.argsort(-degb[nodes_q].sum(1), kind="stable")]
        loads = np.zeros((NBINS, NB), np.int64)
        slots = np.zeros(NBINS, np.int64)
        members = [[] for _ in range(NBINS)]
        for v in nodes_q:
            d = degb[v]
            cand = loads + d
            worst = cand.max(1)
            feas = (slots < NPB) & (cand <= CAP).all(1)
            if feas.any():
                score = np.where(feas, worst * 1000 + slots, 1 << 50)
            else:
                score = np.where(slots < NPB, worst * 1000 + slots, 1 << 50)
            b = int(np.argmin(score))
            members[b].append(v)
            loads[b] += d
            slots[b] += 1
        for b in range(NBINS):
            # bucket q's slot range [q*BKT,(q+1)*BKT) = cores 2q,2q+1
            gbin = q * NBINS + b
            base = gbin * 128
            for i, v in enumerate(members[b]):
                perm[v] = base + i
            all_loads[gbin] = loads[b]
    return perm, all_loads


def _preprocess(x, edge_index, cnt_only=False):
    src = np.asarray(edge_index[0], dtype=np.int64)
    dst = np.asarray(edge_index[1], dtype=np.int64)

    perm, loads = _balance(src, dst)
    cap = max(512, int(-(-loads.max() // 128) * 128))

    gsrc = perm[src]
    gdst = perm[dst]
    core = gdst // SH
    block = (gdst % SH) // 128
    dstloc = (gdst % 128).astype(np.float32)
    bucket = gsrc // BKT
    srcloc = (gsrc % BKT).astype(np.int16)

    pc = block // PIECE
    sp = block % PIECE
    seg = ((core * NPIECE + pc) * NB + bucket) * PIECE + sp
    counts = np.bincount(seg, minlength=NCORES * NB * BLK)
    assert counts.max() <= cap, (counts.max(), cap)

    order = np.argsort(seg, kind="stable")
    seg_s = seg[order]
    starts = np.zeros(NCORES * NB * BLK + 1, np.int64)
    np.cumsum(counts, out=starts[1:])
    pos = np.arange(E, dtype=np.int64) - starts[seg_s]
    slot = seg_s * cap + pos

    total = NCORES * NB * BLK * cap
    idx_arr = np.zeros(total, np.int16)
    idx_arr[slot] = srcloc[order]
    dl_arr = np.full(total, 255.0, np.float32)
    dl_arr[slot] = dstloc[order]
    ts = NB * BLK * cap
    idx_arr = idx_arr.reshape(NCORES, ts)
    dl_arr = dl_arr.reshape(NCORES, ts)

    cnt = np.bincount(gdst, minlength=NP_).astype(np.int32)

    # host-folded deg^{-1/2} scaling of x for layer 1, pre-cast to bf16
    x = np.asarray(x, np.float32)
    dqv = 1.0 / np.sqrt(cnt[perm].astype(np.float64) + 1.0)
    xpad = np.zeros((NP_, D), ml_dtypes.bfloat16)
    xpad[perm] = (x * dqv[:, None].astype(np.float32)).astype(
        ml_dtypes.bfloat16)

    iota = np.tile(np.arange(128, dtype=np.float32),
                   (128, PIECE * cap // 128)).astype(ml_dtypes.bfloat16)
    ident = np.eye(128, dtype=ml_dtypes.bfloat16)

    return cap, perm, idx_arr, dl_arr, cnt, xpad, iota, ident


def kernel(x, W1, b1, W2, b2, edge_index, batch):
    x = np.asarray(x, np.float32)
    W1 = np.asarray(W1, np.float32)
    W2 = np.asarray(W2, np.float32)
    b1 = np.asarray(b1, np.float32)
    b2 = np.asarray(b2, np.float32)

    cap, perm, idx_arr, dl_arr, cnt, xpad, iota, ident = \
        _preprocess(x, edge_index)

    if cap not in _CACHE:
        _CACHE[cap] = _build(cap)
    nc = _CACHE[cap]

    b1t = np.tile(b1, (128, 1))
    b2t = np.tile(b2, (128, 1))
    in_maps = []
    for c in range(NCORES):
        sl = slice(c * SH, (c + 1) * SH)
        wrapped = np.tile(idx_arr[c].reshape(-1, 16).T, (8, 1))
        in_maps.append({
            "xT": np.ascontiguousarray(xpad[sl].T),
            "w1": W1, "w2": W2, "b1t": b1t, "b2t": b2t,
            "cnt": np.ascontiguousarray(cnt[sl].reshape(BLK, 128).T),
            "sqd": np.ascontiguousarray(
                np.sqrt(cnt[sl][None, :].astype(np.float64) + 1.0)
                .astype(ml_dtypes.bfloat16)),
            "idx": np.ascontiguousarray(wrapped),
            "dstloc": np.ascontiguousarray(
                dl_arr[c].reshape(-1, 128).T.astype(ml_dtypes.bfloat16)),
            "iota": iota, "ident": ident,
        })

    import os
    trace = bool(os.environ.get("KERNEL_TRACE"))
    rr = run_bass_kernel_spmd(nc, in_maps, list(range(NCORES)), trace=trace)
    if trace:
        kernel.last_results = rr
    # out[p, c*128+f] per core -> rows c*128+p
    outs = []
    for c in range(NCORES):
        o = rr.results[c]["out"].reshape(128, BLK, 128)
        outs.append(np.transpose(o, (1, 0, 2)).reshape(SH, 128))
    out = np.concatenate(outs, axis=0)
    return np.ascontiguousarray(out[perm])


# revision 26
# speedup vs baseline: 1.0311x; 1.0167x over previous
"""Two-layer GCN (PyG GCNConv defaults) on 8 Trainium2 NeuronCores.

Strategy (graph/data parallel, per sharding hint):
  - Destination nodes sharded 8 x 12800 (N=100000 padded to 102400), with a
    host-side balanced assignment of nodes to (core, block) bins so that
    every (dst-block, src-bucket) segment holds <= cap (target 512) edges.
    Buckets are within-shard quarters (25 blocks each); nodes are kept inside
    their home bucket's block range so src bucket ids are known before the
    permutation exists.  The output is un-permuted on the host.
  - Per layer: g = deg^{-1/2} * (h @ W) computed on each core's own shard
    (layer 1 folds deg^{-1/2} into x on the host, x pre-cast to bf16), then
    AllGather'd per bucket-quarter (4 chunked collectives, bf16) so each
    gather queue can start as soon as its bucket table lands.
  - Edge aggregation per core: edges with dst in the core's shard, sorted by
    (piece, src-bucket, dst-block).  Source features fetched with dma_gather
    (int16 indices into the 25600-row bucket tables); the 4 buckets map to
    the 4 SWDGE queues with 4 pieces in flight per queue so all four queues'
    descriptor pipelines stay saturated (the gather is descriptor-rate-bound
    at ~7.9ns/desc/queue).
  - segment_sum realized as selector matmuls: S.T[e, d] = (dstloc[e] == d)
    built on-chip via is_equal against an iota tile, one batched is_equal per
    (piece, bucket); each dst block's four bucket-segments accumulate in one
    PSUM slice, finalized in place:
    out = deg^{-1/2} * (segsum + g_own) + b, LeakyReLU(0.01).  Layer 2 reuses
    the same edge structure against the layer-2 g tables.
  - The bounce buffers and the output are partition-major so every per-piece
    DMA is a contiguous run per partition (no 256B-descriptor storms
    competing with the SWDGE gather descriptors).
"""
import sys

sys.path.insert(0, "/opt/trn_rl_repo")

import numpy as np
import ml_dtypes

import concourse.bacc as bacc
import concourse.mybir as mybir
import concourse.tile as tile
from concourse.bass_utils import run_bass_kernel_spmd

NCORES = 8
N = 100000
E = 1600000
D = 128
BLK = 100               # dst blocks per core
SH = BLK * 128          # dst shard per core (12800)
NP_ = SH * NCORES       # padded node count 102400
NB = 4                  # src buckets = within-shard quarters (25 blocks)
QB = BLK // NB          # blocks per bucket-quarter (25)
BKT = NP_ // NB         # 25600 rows per bucket table
PIECE = 5               # dst blocks per dma_gather piece (100 = 20 * 5)
NPIECE = BLK // PIECE   # 20 pieces
NEG = 0.01

fp32 = mybir.dt.float32
bf16 = mybir.dt.bfloat16
i16 = mybir.dt.int16
i32 = mybir.dt.int32

_CACHE = {}


def _build(cap):
    ck = cap // 128                 # chunks per segment
    ts = NB * BLK * cap             # idx stream length per core / layer
    nch = ts // 128                 # total chunks in stream
    pidx = PIECE * cap              # idxs per gather piece per bucket
    pck = PIECE * ck                # chunks per (piece, bucket)

    nc = bacc.Bacc("TRN2", num_devices=NCORES, num_swdge_queues=4,
                   dynamic_dma_scratch_size=32768)
    xT_in = nc.dram_tensor("xT", [128, SH], bf16, kind="ExternalInput")
    w1_in = nc.dram_tensor("w1", [128, 128], fp32, kind="ExternalInput")
    w2_in = nc.dram_tensor("w2", [128, 128], fp32, kind="ExternalInput")
    b1_in = nc.dram_tensor("b1t", [128, 128], fp32, kind="ExternalInput")
    b2_in = nc.dram_tensor("b2t", [128, 128], fp32, kind="ExternalInput")
    cnt_in = nc.dram_tensor("cnt", [128, BLK], i32, kind="ExternalInput")
    sqd_in = nc.dram_tensor("sqd", [1, SH], bf16, kind="ExternalInput")
    idx_in = nc.dram_tensor("idx", [128, ts // 16], i16, kind="ExternalInput")
    dl_in = nc.dram_tensor("dstloc", [128, nch], bf16, kind="ExternalInput")
    iota_in = nc.dram_tensor("iota", [128, PIECE * cap], bf16,
                             kind="ExternalInput")
    ident_in = nc.dram_tensor("ident", [128, 128], bf16, kind="ExternalInput")
    # partition-major output: out2[p, c*128 + f] = out[c*128 + p, f]
    out_t = nc.dram_tensor("out", [128, SH], fp32, kind="ExternalOutput")

    with tile.TileContext(nc) as tc:
        with (
            tc.tile_pool(name="const", bufs=1) as cpool,
            tc.tile_pool(name="xchunk", bufs=3) as xpool,
            tc.tile_pool(name="msg", bufs=4) as mpool,
            tc.tile_pool(name="st", bufs=2) as stpool,
            tc.tile_pool(name="fin", bufs=3) as fpool,
            tc.tile_pool(name="ps_a", bufs=2, space="PSUM") as ps_a,
            tc.tile_pool(name="ps_w", bufs=1, space="PSUM") as ps_w,
            tc.tile_pool(name="ps_seg", bufs=4, space="PSUM") as ps_seg,
            tc.tile_pool(name="ps_t", bufs=1, space="PSUM") as ps_t,
            tc.tile_pool(name="dram", bufs=1, space="DRAM") as dram,
        ):
            # ---- resident constants -------------------------------------
            w1f = cpool.tile([128, 128], fp32)
            nc.sync.dma_start(w1f[:], w1_in[:])
            w1b = cpool.tile([128, 128], bf16)
            nc.vector.tensor_copy(w1b[:], w1f[:])
            w2f = cpool.tile([128, 128], fp32)
            nc.sync.dma_start(w2f[:], w2_in[:])
            w2b = cpool.tile([128, 128], bf16)
            nc.vector.tensor_copy(w2b[:], w2f[:])
            b1t = cpool.tile([128, 128], fp32)
            nc.sync.dma_start(b1t[:], b1_in[:])
            b2t = cpool.tile([128, 128], fp32)
            nc.sync.dma_start(b2t[:], b2_in[:])
            iota = cpool.tile([128, PIECE * cap], bf16)
            nc.sync.dma_start(iota[:], iota_in[:])
            ident = cpool.tile([128, 128], bf16)
            nc.sync.dma_start(ident[:], ident_in[:])
            dstloc = cpool.tile([128, nch], bf16)
            nc.sync.dma_start(dstloc[:], dl_in[:])
            # full idx stream resident (shared by both layers)
            ixall = cpool.tile([128, ts // 16], i16)
            nc.sync.dma_start(ixall[:], idx_in[:])

            # deg^{-1/2} from int32 counts:  1 / sqrt(cnt + 1)
            cnts = cpool.tile([128, BLK], i32)
            nc.sync.dma_start(cnts[:], cnt_in[:])
            degf = cpool.tile([128, BLK], fp32)
            nc.vector.tensor_copy(degf[:], cnts[:])
            sq = cpool.tile([128, BLK], fp32)
            nc.scalar.activation(sq[:], degf[:],
                                 mybir.ActivationFunctionType.Sqrt, bias=1.0)
            dq = cpool.tile([128, BLK], fp32)
            nc.vector.reciprocal(dq[:], sq[:])

            b1r = cpool.tile([1, 128], bf16)
            nc.vector.tensor_copy(b1r[:], b1t[0:1, :])
            b2r = cpool.tile([1, 128], bf16)
            nc.vector.tensor_copy(b2r[:], b2t[0:1, :])

            # DRAM bounce (partition-major, one per bucket-quarter) + tables
            g1_bq = [dram.tile([128, QB * 128], bf16, name=f"g1_bq{r}")
                     for r in range(NB)]
            g2_bq = [dram.tile([128, QB * 128], bf16, name=f"g2_bq{r}")
                     for r in range(NB)]
            g1_t = [dram.tile([BKT, D], bf16, name=f"g1_t{r}",
                              addr_space="Shared") for r in range(NB)]
            g2_t = [dram.tile([BKT, D], bf16, name=f"g2_t{r}",
                              addr_space="Shared") for r in range(NB)]

            def cc(src_bq, dst_t):
                # AllGather output is the raw core-major concat of the 8
                # [128, QB*128] partition-major shard buffers; viewed as a
                # [BKT, 128] table, row (c*128+p)*QB + a holds the features
                # of (core c, quarter-block a, slot p).
                nc.gpsimd.collective_compute(
                    "AllGather", mybir.AluOpType.bypass,
                    replica_groups=[list(range(NCORES))],
                    ins=[src_bq[:].opt()],
                    outs=[dst_t[:].opt()])

            # ---- phase A: g1 = (dq*x) @ W1 on own shard (dq host-folded)
            with nc.named_scope("phaseA"):
                for r in range(NB):
                    for i in range(0, QB, 4):
                        w = min(4, QB - i)
                        c0 = (r * QB + i) * 128
                        xc = xpool.tile([128, 512], bf16, tag="xc")
                        nc.sync.dma_start(xc[:, :w * 128],
                                          xT_in[:, c0:c0 + w * 128])
                        psA = ps_a.tile([128, 512], fp32, space="PSUM",
                                        tag="psA")
                        for j in range(w):
                            nc.tensor.matmul(
                                out=psA[:, j * 128:(j + 1) * 128],
                                lhsT=xc[:, j * 128:(j + 1) * 128],
                                rhs=w1b[:], start=True, stop=True)
                        gtb = xpool.tile([128, 512], bf16, tag="gtb")
                        nc.scalar.activation(
                            gtb[:, :w * 128], psA[:, :w * 128],
                            mybir.ActivationFunctionType.Copy)
                        nc.sync.dma_start(
                            g1_bq[r][:, i * 128:(i + w) * 128],
                            gtb[:, :w * 128])
                    with nc.named_scope(f"cc1_{r}"):
                        cc(g1_bq[r], g1_t[r])

            # ---- aggregation over edges (shared for both layers) --------
            def aggregate(g_t, g_bq, b_row, finalize, stage_dt, flush,
                          after_piece):
                for p in range(NPIECE):
                    msgs = []
                    for r in range(NB):
                        i0 = (p * NB + r) * pidx // 16
                        msg = mpool.tile([128, pck, 128], bf16, tag=f"msg{r}")
                        nc.gpsimd.dma_gather(
                            msg[:], g_t[r][:, :],
                            ixall[:, i0:i0 + pidx // 16],
                            pidx, pidx, 128, single_packet=False,
                            queue_num=r)
                        msgs.append(msg)
                    # own-shard rows for the self-loop term
                    gown = fpool.tile([128, PIECE * 128], bf16, tag="gown")
                    nc.sync.dma_start(
                        gown[:],
                        g_bq[p // PIECE][:, (p % PIECE) * PIECE * 128:
                                         (p % PIECE + 1) * PIECE * 128])
                    # per-piece sqrt(deg) row for the rank-1 bias fold
                    sqp = fpool.tile([1, PIECE * 128], bf16, tag="sqp")
                    nc.sync.dma_start(
                        sqp[:],
                        sqd_in[0:1, p * PIECE * 128:(p + 1) * PIECE * 128])
                    # batched selector build: one is_equal per (piece,bucket)
                    sts = []
                    for r in range(NB):
                        gc0 = (p * NB + r) * pck
                        st = stpool.tile([128, pck, 128], bf16, tag=f"st{r}")
                        nc.vector.tensor_tensor(
                            st[:],
                            iota[:].rearrange("q (c j) -> q c j", c=pck),
                            dstloc[:, gc0:gc0 + pck].to_broadcast(
                                [128, pck, 128]),
                            mybir.AluOpType.is_equal)
                        sts.append(st)
                    stage = fpool.tile([128, PIECE * 128], stage_dt,
                                       tag="stage")
                    for s in range(PIECE):
                        c = p * PIECE + s
                        pseg = ps_seg.tile([128, 128], fp32, space="PSUM",
                                           tag="pseg")
                        for r in range(NB):
                            for k in range(ck):
                                nc.tensor.matmul(
                                    out=pseg[:],
                                    lhsT=sts[r][:, s * ck + k, :],
                                    rhs=msgs[r][:, s * ck + k, :],
                                    start=(r == 0 and k == 0),
                                    stop=False)
                        # self-loop: pseg += I.T @ g_own ;  bias (pre-scale):
                        # pseg += sqrt(deg) (x) b  so that (pseg)*dq = out+b
                        nc.tensor.matmul(out=pseg[:], lhsT=ident[:],
                                         rhs=gown[:, s * 128:(s + 1) * 128],
                                         start=False, stop=False)
                        nc.tensor.matmul(out=pseg[:],
                                         lhsT=sqp[0:1, s * 128:(s + 1) * 128],
                                         rhs=b_row[:],
                                         start=False, stop=True)
                        finalize(c, pseg[:], stage[:, s * 128:(s + 1) * 128])
                    flush(p, stage)
                    after_piece(p)

            # ---- layer-1 finalize: h2 path, produces g2 -----------------
            def fin1(c, pslice, sl):
                r1 = fpool.tile([128, 128], bf16, tag="r1")
                nc.scalar.activation(r1[:], pslice,
                                     mybir.ActivationFunctionType.Lrelu,
                                     scale=dq[:, c:c + 1], alpha=NEG)
                ptr = ps_t.tile([128, 128], bf16, space="PSUM", tag="ptr")
                nc.tensor.transpose(ptr[:], r1[:], ident[:])
                r1T = fpool.tile([128, 128], bf16, tag="r1T")
                nc.vector.tensor_copy(r1T[:], ptr[:])
                ph2 = ps_w.tile([128, 128], fp32, space="PSUM", tag="ph")
                nc.tensor.matmul(out=ph2[:], lhsT=r1T[:], rhs=w2b[:],
                                 start=True, stop=True)
                nc.scalar.activation(sl, ph2[:],
                                     mybir.ActivationFunctionType.Copy,
                                     scale=dq[:, c:c + 1])

            def flush1(p, stage):
                nc.sync.dma_start(
                    g2_bq[p // PIECE][:, (p % PIECE) * PIECE * 128:
                                      (p % PIECE + 1) * PIECE * 128],
                    stage[:])

            # layer-2 chunk collectives: emitted at the end of agg1; their
            # dispatches run ahead (issue-ahead) and overlap agg1's tail.
            def after1(p):
                if p == NPIECE - 1:
                    for r in range(NB):
                        with nc.named_scope(f"cc2_{r}"):
                            cc(g2_bq[r], g2_t[r])

            def fin2(c, pslice, sl):
                nc.scalar.activation(sl, pslice,
                                     mybir.ActivationFunctionType.Lrelu,
                                     scale=dq[:, c:c + 1], alpha=NEG)

            def flush2(p, stage):
                nc.sync.dma_start(
                    out_t[:, p * PIECE * 128:(p + 1) * PIECE * 128], stage[:])

            def after2(p):
                pass

            with nc.named_scope("agg1"):
                aggregate(g1_t, g1_bq, b1r, fin1, bf16, flush1, after1)

            with nc.named_scope("agg2"):
                aggregate(g2_t, g2_bq, b2r, fin2, fp32, flush2, after2)

    nc.compile()
    return nc


def _balance(src, dst):
    """Choose the node -> global-slot permutation.

    1. Home buckets: partition nodes into NB groups (<= BKT nodes each)
       equalizing total OUT-edge counts.
    2. Per home bucket, greedily pack its nodes into its 200 bins (the
       bucket's QB blocks on each of the 8 cores, 128 slots each), balancing
       the bins' per-src-bucket IN-edge loads with a hard cap of 512.
    Returns (perm, loads) where perm[v] = global slot.
    """
    outdeg = np.bincount(src, minlength=N).astype(np.int64)
    order = np.argsort(-outdeg, kind="stable")
    bket = np.full(N, -1, np.int8)
    nodes_per_bucket = N // NB
    for r in range(NB):
        bket[order[r::NB][:nodes_per_bucket]] = r

    bucket_of_src = bket[src]
    degb = np.zeros((N, NB), np.int64)
    np.add.at(degb, (dst, bucket_of_src), 1)

    CAP = 512
    NPB = 128
    NBINS = NCORES * QB  # 200 bins per bucket group
    perm = np.full(N, -1, np.int64)
    all_loads = np.zeros((NCORES * BLK, NB), np.int64)
    for q in range(NB):
        nodes_q = np.where(bket == q)[0]
        nodes_q = nodes_q[np.argsort(-degb[nodes_q].sum(1), kind="stable")]
        loads = np.zeros((NBINS, NB), np.int64)
        slots = np.zeros(NBINS, np.int64)
        members = [[] for _ in range(NBINS)]
        for v in nodes_q:
            d = degb[v]
            cand = loads + d
            worst = cand.max(1)
            feas = (slots < NPB) & (cand <= CAP).all(1)
            if feas.any():
                score = np.where(feas, worst * 1000 + slots, 1 << 50)
            else:
                score = np.where(slots < NPB, worst * 1000 + slots, 1 << 50)
            b = int(np.argmin(score))
            members[b].append(v)
            loads[b] += d
            slots[b] += 1
        for b in range(NBINS):
            core, j = divmod(b, QB)
            gbin = core * BLK + q * QB + j
            base = gbin * 128
            for i, v in enumerate(members[b]):
                perm[v] = base + i
            all_loads[gbin] = loads[b]
    return perm, all_loads


def _preprocess(x, edge_index):
    src = np.asarray(edge_index[0], dtype=np.int64)
    dst = np.asarray(edge_index[1], dtype=np.int64)

    perm, loads = _balance(src, dst)
    cap = max(512, int(-(-loads.max() // 128) * 128))

    gsrc = perm[src]
    gdst = perm[dst]
    core = gdst // SH
    block = (gdst % SH) // 128
    dstloc = (gdst % 128).astype(np.float32)
    bucket = (gsrc % SH) // (QB * 128)
    # bucket table row: cores-major [8*128, QB, 128]-bf16 layout
    srcloc = ((gsrc // SH) * 128 + gsrc % 128) * QB \
        + ((gsrc % SH) // 128) % QB
    srcloc = srcloc.astype(np.int16)

    pc = block // PIECE
    sp = block % PIECE
    seg = ((core * NPIECE + pc) * NB + bucket) * PIECE + sp
    counts = np.bincount(seg, minlength=NCORES * NB * BLK)
    assert counts.max() <= cap, (counts.max(), cap)

    order = np.argsort(seg, kind="stable")
    seg_s = seg[order]
    starts = np.zeros(NCORES * NB * BLK + 1, np.int64)
    np.cumsum(counts, out=starts[1:])
    pos = np.arange(E, dtype=np.int64) - starts[seg_s]
    slot = seg_s * cap + pos

    total = NCORES * NB * BLK * cap
    idx_arr = np.zeros(total, np.int16)
    idx_arr[slot] = srcloc[order]
    dl_arr = np.full(total, 255.0, np.float32)
    dl_arr[slot] = dstloc[order]
    ts = NB * BLK * cap
    idx_arr = idx_arr.reshape(NCORES, ts)
    dl_arr = dl_arr.reshape(NCORES, ts)

    cnt = np.bincount(gdst, minlength=NP_).astype(np.int32)

    # host-folded deg^{-1/2} scaling of x for layer 1, pre-cast to bf16
    x = np.asarray(x, np.float32)
    dqv = 1.0 / np.sqrt(cnt[perm].astype(np.float64) + 1.0)
    xpad = np.zeros((NP_, D), ml_dtypes.bfloat16)
    xpad[perm] = (x * dqv[:, None].astype(np.float32)).astype(
        ml_dtypes.bfloat16)

    iota = np.tile(np.arange(128, dtype=np.float32),
                   (128, PIECE * cap // 128)).astype(ml_dtypes.bfloat16)
    ident = np.eye(128, dtype=ml_dtypes.bfloat16)

    return cap, perm, idx_arr, dl_arr, cnt, xpad, iota, ident


def kernel(x, W1, b1, W2, b2, edge_index, batch):
    x = np.asarray(x, np.float32)
    W1 = np.asarray(W1, np.float32)
    W2 = np.asarray(W2, np.float32)
    b1 = np.asarray(b1, np.float32)
    b2 = np.asarray(b2, np.float32)

    cap, perm, idx_arr, dl_arr, cnt, xpad, iota, ident = \
        _preprocess(x, edge_index)

    if cap not in _CACHE:
        _CACHE[cap] = _build(cap)
    nc = _CACHE[cap]

    b1t = np.tile(b1, (128, 1))
    b2t = np.tile(b2, (128, 1))
    in_maps = []
    for c in range(NCORES):
        sl = slice(c * SH, (c + 1) * SH)
        wrapped = np.tile(idx_arr[c].reshape(-1, 16).T, (8, 1))
        in_maps.append({
            "xT": np.ascontiguousarray(xpad[sl].T),
            "w1": W1, "w2": W2, "b1t": b1t, "b2t": b2t,
            "cnt": np.ascontiguousarray(cnt[sl].reshape(BLK, 128).T),
            "sqd": np.ascontiguousarray(
                np.sqrt(cnt[sl][None, :].astype(np.float64) + 1.0)
                .astype(ml_dtypes.bfloat16)),
            "idx": np.ascontiguousarray(wrapped),
            "dstloc": np.ascontiguousarray(
                dl_arr[c].reshape(-1, 128).T.astype(ml_dtypes.bfloat16)),
            "iota": iota, "ident": ident,
        })

    import os
    trace = bool(os.environ.get("KERNEL_TRACE"))
    rr = run_bass_kernel_spmd(nc, in_maps, list(range(NCORES)), trace=trace)
    if trace:
        kernel.last_results = rr
    # out[p, c*128+f] per core -> rows c*128+p
    outs = []
    for c in range(NCORES):
        o = rr.results[c]["out"].reshape(128, BLK, 128)
        outs.append(np.transpose(o, (1, 0, 2)).reshape(SH, 128))
    out = np.concatenate(outs, axis=0)
    return np.ascontiguousarray(out[perm])
